# revision 20
# baseline (speedup 1.0000x reference)
"""Grouped per-filter conv (64 groups, 3x3x64 -> 1) + TFLite requant, 8 trn2 cores.

Sharding: filter dim F=64 split 8 groups/core (embarrassingly parallel).

Per-core pipeline (v2 — single-pass tap matmul + two selector merge passes):
  host:  x[8,256,256,64] i8 -> chan-major bf16 xt[4,128,65536]
         (pair p, partition 64h+o <-> local group g = 2p+h)
  PE  stage1 (per 512-px chunk c, per pair p): one matmul
         psum1[32p + 9h+3m+n, i] = sum_o xt[p,64h+o,512c+i] * w[2p+h][m,n,o]
  DVE evac1: psum1 -> praw strip buffer (bf16), + 2-col tail for the n-shift
  PE  passA (n-merge): 3 accumulating selector matmuls with rhs col offsets n:
         psum2[6p+3h+m, i] = U[(g,m), 512c+i] = sum_n praw[.., 512c+i+n]
  ACT evac2: psum2 -> Usb strip buffer (bf16), +512-col mirror for m-shifts
  PE  passB (m-merge): 3 accumulating selector matmuls with rhs offsets 256m:
         psum3[32(c%4) + g, i] = acc[g, 512c+i] = sum_m U[(g,m), 512c+i+256m]
  DVE requant (per 4 chunks): res = clip(rne((acc+bias)*RED_M/2^26) + zp)
  DMA out[g, 2 rows, 0:254] per chunk
"""

import numpy as np
import ml_dtypes

F, H, W, CIN = 64, 256, 256, 64
KH = KW = 3
HO = WO = H - KH + 1  # 254
NCORES = 8
GPC = F // NCORES  # 8 groups per core
NPIX = H * W  # 65536
NCHUNK = NPIX // 512  # 128
SPC = 8                # chunks per strip
NSTRIP = NCHUNK // SPC  # 16
SQ = SPC * 512         # 4096 strip pixels
Q_MANTISSA = 1340958551
EXPONENT = -11
ZP = -3
RED_M = (Q_MANTISSA + (1 << 15)) >> 16 if Q_MANTISSA < 2147418112 else 32767
TOTAL_SHIFTS = 15 - EXPONENT  # 26
C_SCALE = float(RED_M) / float(1 << TOTAL_SHIFTS)

_CACHE = {}


def _patch_drain(tile_mod):
    """Split multi-sem-wait instructions: the walrus in this container rejects
    >1 sync-wait per instruction, so park extra waits on preceding NOPs/waits
    on the same engine (program order preserves the happens-before)."""
    if getattr(tile_mod.TileContext, "_drain_wait_split", False):
        return

    def _drain_and_barrier(self, tick_clock, wait_clock):
        nc = self.nc
        probe = nc.sync.nop()
        wait_clock.add_sem_waits(
            probe.ins, tile_mod.ScopedClock({None: tick_clock.global_clock}))
        waits = list(probe.ins.sync_info.on_wait or [])
        if len(waits) > 1:
            allocated = {s.name: s for s in self.sems.allocated().values()}
            probe.ins.sync_info.on_wait = [waits[0]]
            for wcond in waits[1:]:
                h = allocated[wcond.ant_name]
                assert wcond.wait_mode == "sem-ge-imm", wcond
                nc.sync.wait_ge(h, wcond.wait_value)
        nc.sync.drain()
        nc.all_engine_barrier()
        assert self.sems is not None
        popped = nc._tile_sem_poison_stack.pop()
        assert popped is self._sem_poison
        nc.clear_and_free_semaphores(list(self.sems.allocated().values()))
        nc.all_engine_barrier()

    tile_mod.TileContext._drain_and_barrier = _drain_and_barrier

    import concourse.mybir as mybir

    _TPB_ENGINES = {
        mybir.EngineType.PE, mybir.EngineType.DVE, mybir.EngineType.Activation,
        mybir.EngineType.SP, mybir.EngineType.Pool,
    }
    orig_lower = tile_mod.TileContext._lower_ordered_insts

    def _lower_ordered_insts(self, ordered):
        nc = self.nc
        for bb_name, insts in ordered.items():
            out = []
            for inst in insts:
                si = inst.sync_info
                if (si is not None and si.on_wait and len(si.on_wait) > 1
                        and inst.engine in _TPB_ENGINES):
                    waits = list(si.on_wait)
                    for wcond in waits[:-1]:
                        nop = mybir.InstNoOp(name=nc.get_next_instruction_name())
                        nop.engine = inst.engine
                        nop.sync_info = mybir.SyncInfo(on_wait=[wcond], on_update=[])
                        out.append(nop)
                    si.on_wait = [waits[-1]]
                out.append(inst)
            insts[:] = out
        return orig_lower(self, ordered)

    tile_mod.TileContext._lower_ordered_insts = _lower_ordered_insts
    tile_mod.TileContext._drain_wait_split = True


def _build_bass():
    from concourse import bass, mybir
    from concourse import tile as tile_mod
    from concourse.tile import TileContext

    _patch_drain(tile_mod)
    dt = mybir.dt
    Alu = mybir.AluOpType
    nc = bass.Bass("TRN2", target_bir_lowering=False, debug=False,
                   num_devices=NCORES)

    xt = nc.dram_tensor("xt", [4, 128, NPIX], dt.int8, kind="ExternalInput")
    wt9 = nc.dram_tensor("wt9", [4, 128, 32], dt.bfloat16, kind="ExternalInput")
    selA = nc.dram_tensor("selA", [128, 96], dt.bfloat16, kind="ExternalInput")
    selB = nc.dram_tensor("selB", [24, 96], dt.bfloat16, kind="ExternalInput")
    biasv = nc.dram_tensor("biasv", [128, 1], dt.float32, kind="ExternalInput")
    out = nc.dram_tensor("out", [GPC, HO, WO], dt.int8, kind="ExternalOutput")
    out_ap = out.ap()

    with TileContext(nc) as tc:
        with (
            tc.tile_pool(name="xbuf", bufs=2) as xpool,
            tc.tile_pool(name="consts", bufs=1) as cpool,
            tc.tile_pool(name="pers", bufs=1) as ppool,
            tc.tile_pool(name="psum1", bufs=3, space="PSUM") as ps1pool,
            tc.tile_pool(name="psum2", bufs=2, space="PSUM") as ps2pool,
            tc.tile_pool(name="psum3", bufs=2, space="PSUM") as ps3pool,
            tc.tile_pool(name="stg", bufs=2) as stpool,
            tc.tile_pool(name="t2", bufs=2) as t2pool,
            tc.tile_pool(name="res", bufs=2) as respool,
        ):
            # ---- constants ----
            wt_sb = []
            for p in range(4):
                wtp = cpool.tile([128, 32], dt.bfloat16, tag=f"wt{p}", name=f"wt{p}")
                nc.sync.dma_start(out=wtp[:], in_=wt9.ap()[p])
                wt_sb.append(wtp)
            selA_sb = cpool.tile([128, 96], dt.bfloat16, tag="selA")
            nc.sync.dma_start(out=selA_sb[:], in_=selA.ap()[:])
            selB_sb = cpool.tile([24, 96], dt.bfloat16, tag="selB")
            nc.sync.dma_start(out=selB_sb[:], in_=selB.ap()[:])
            bias_sb = cpool.tile([128, 1], dt.float32, tag="bias")
            nc.sync.dma_start(out=bias_sb[:], in_=biasv.ap()[:])

            # persistent strip buffers
            praw = ppool.tile([128, SQ + 2], dt.bfloat16, tag="praw")
            usb = ppool.tile([24, SQ + 512], dt.bfloat16, tag="usb")

            # strip x buffers: dict strip -> 4 tiles
            xbufs = {}

            def load_strip(s):
                if s >= NSTRIP or s in xbufs:
                    return
                tiles = []
                # casting DMAs (int8 HBM -> bf16 SBUF); only gpsimd can cast.
                # strip 0 in quarter-strip pieces for startup latency.
                npiece = 4 if s == 0 else (2 if s == 1 else 1)
                pq = SQ // npiece
                for p in range(4):
                    xb = xpool.tile([128, SQ], dt.bfloat16, tag=f"xb{p}")
                    for i in range(npiece):
                        nc.gpsimd.dma_start(
                            out=xb[:, i * pq:(i + 1) * pq],
                            in_=xt.ap()[p, :, s * SQ + i * pq:s * SQ + (i + 1) * pq])
                    tiles.append(xb)
                xbufs[s] = tiles

            load_strip(0)
            load_strip(1)
            load_strip(2)

            ps3 = None
            NSUP = NCHUNK // 2  # super-chunks of 2 chunks (shared lhsT loads)
            # software-pipelined loop: stage1(C) | passA(C-2) | passB(C-4)
            for C in range(NSUP + 4):
                if C < NSUP:
                    c0 = 2 * C
                    s, cl0 = divmod(c0, SPC)
                    if cl0 == 0:
                        load_strip(s + 2)
                    # ---- stage1: 8 matmuls, lhsT-major over 2 chunks ----
                    ps1s = [ps1pool.tile([128, 512], dt.float32, tag="ps1",
                                         name=f"ps1_{C}_{j}") for j in range(2)]
                    for p in range(4):
                        for j in range(2):
                            nc.tensor.matmul(
                                out=ps1s[j][32 * p:32 * p + 32, :],
                                lhsT=wt_sb[p][:, 0:32],
                                rhs=xbufs[s][p][:, 512 * (cl0 + j):512 * (cl0 + j) + 512],
                                start=True, stop=True,
                                tile_position=(0, 32 * p),
                            )
                    # ---- evac1 (DVE) -> praw ----
                    for j in range(2):
                        cl = cl0 + j
                        nc.vector.tensor_copy(out=praw[:, 512 * cl:512 * cl + 512],
                                              in_=ps1s[j][:, :])
                        if cl == 0 and c0 > 0:
                            # tail cols for previous strip's last passA window
                            nc.vector.tensor_copy(out=praw[:, SQ:SQ + 2],
                                                  in_=ps1s[j][:, 0:2])
                    if cl0 == 0 and c0 >= SPC:
                        xbufs.pop(c0 // SPC - 1, None)

                # ---- passA for super-chunk A = C-2 ----
                A = C - 2
                if 0 <= A < NSUP:
                    a0 = 2 * A
                    ps2s = [ps2pool.tile([24, 512], dt.float32, tag="ps2",
                                         name=f"ps2_{A}_{j}") for j in range(2)]
                    for n in range(3):
                        for j in range(2):
                            al = (a0 + j) % SPC
                            nc.tensor.matmul(
                                out=ps2s[j][0:24, :],
                                lhsT=selA_sb[:, 32 * n:32 * n + 24],
                                rhs=praw[:, 512 * al + n:512 * al + n + 512],
                                start=(n == 0), stop=(n == 2),
                                tile_position=(0, 0),
                            )
                    # ---- evac2 (ACT) -> usb ----
                    for j in range(2):
                        a = a0 + j
                        al = a % SPC
                        nc.scalar.copy(out=usb[:, 512 * al:512 * al + 512],
                                       in_=ps2s[j][0:24, :])
                        if al == 0 and a > 0:
                            # mirror for previous strip's passB m-shift windows
                            nc.scalar.copy(out=usb[:, SQ:SQ + 512],
                                           in_=ps2s[j][0:24, :])

                # ---- passB for super-chunk B = C-4 ----
                B = C - 4
                if 0 <= B < NSUP:
                    b0 = 2 * B
                    k0 = b0 % 4
                    if k0 == 0:
                        ps3 = ps3pool.tile([128, 512], dt.float32, tag="ps3")
                    for m in range(3):
                        for j in range(2):
                            b = b0 + j
                            bl = b % SPC
                            k = k0 + j
                            nc.tensor.matmul(
                                out=ps3[32 * k:32 * k + 32, :],
                                lhsT=selB_sb[0:24, 32 * m:32 * m + 32],
                                rhs=usb[0:24,
                                        512 * bl + 256 * m:512 * bl + 256 * m + 512],
                                start=(m == 0), stop=(m == 2),
                                tile_position=(0, 32 * k),
                            )
                    if k0 == 2:
                        t = b0 // 4
                        # ---- requant (DVE), float path with RNE convert ----
                        stg = stpool.tile([128, 512], dt.float32, tag="stg")
                        nc.vector.tensor_scalar(stg[:, :], ps3[:, :],
                                                bias_sb[:, 0:1], C_SCALE,
                                                Alu.add, Alu.mult)
                        t2 = t2pool.tile([128, 512], dt.float32, tag="t2")
                        nc.vector.tensor_scalar(t2[:, :], stg[:, :],
                                                -125.49, 130.49,
                                                Alu.max, Alu.min)
                        res = respool.tile([128, 512], dt.int8, tag="res")
                        nc.vector.tensor_scalar(res[:, :], t2[:, :],
                                                float(ZP), None, Alu.add)
                        # ---- out DMA per chunk (skip invalid chunk 127) ----
                        res_v = res.rearrange("v (r cc) -> v r cc", r=2)
                        for kk in range(4):
                            bb = 4 * t + kk
                            if bb >= NCHUNK - 1:
                                continue
                            nc.sync.dma_start(
                                out=out_ap[:, 2 * bb:2 * bb + 2, 0:WO],
                                in_=res_v[32 * kk:32 * kk + 8, :, 0:WO],
                            )
    return nc


def _host_prep(x, w, bias, core):
    """Build per-core input arrays (local group g = 2p + h)."""
    bf16 = ml_dtypes.bfloat16
    g0 = core * GPC
    xs = x[g0:g0 + GPC]  # [8,256,256,64] int8
    xsq = np.ascontiguousarray(
        xs.reshape(GPC, NPIX, CIN).transpose(0, 2, 1))  # [8,64,65536] int8
    xt = np.empty([4, 128, NPIX], dtype=np.int8)
    for p in range(4):
        xt[p, 0:64] = xsq[2 * p]
        xt[p, 64:128] = xsq[2 * p + 1]

    ws = w[g0:g0 + GPC].astype(np.float32)  # [8,3,3,64]
    wt9 = np.zeros([4, 128, 32], dtype=bf16)
    for p in range(4):
        for h in range(2):
            for m in range(3):
                for n in range(3):
                    wt9[p, 64 * h:64 * h + 64, 9 * h + 3 * m + n] = \
                        ws[2 * p + h, m, n, :]

    selA = np.zeros([128, 96], dtype=bf16)
    for p in range(4):
        for h in range(2):
            for m in range(3):
                for n in range(3):
                    selA[32 * p + 9 * h + 3 * m + n, 32 * n + 6 * p + 3 * h + m] = 1.0

    selB = np.zeros([24, 96], dtype=bf16)
    for p in range(4):
        for h in range(2):
            for m in range(3):
                selB[6 * p + 3 * h + m, 32 * m + 2 * p + h] = 1.0

    bv = np.zeros([128, 1], dtype=np.float32)
    for k in range(4):
        bv[32 * k:32 * k + GPC, 0] = bias[g0:g0 + GPC].astype(np.float32)

    return {"xt": xt, "wt9": wt9, "selA": selA, "selB": selB, "biasv": bv}


def kernel(x, w, bias, q_mantissa, exponent, output_zero_point):
    from concourse.bass_utils import run_bass_kernel_spmd

    x = np.asarray(x)
    w = np.asarray(w)
    bias = np.asarray(bias)
    assert int(q_mantissa) == Q_MANTISSA and int(exponent) == EXPONENT \
        and int(output_zero_point) == ZP, "requant params are hardcoded"

    if "nc" not in _CACHE:
        _CACHE["nc"] = _build_bass()
    nc = _CACHE["nc"]

    in_maps = [_host_prep(x, w, bias, c) for c in range(NCORES)]
    res = run_bass_kernel_spmd(nc, in_maps, list(range(NCORES)))
    outs = [res.results[c]["out"] for c in range(NCORES)]
    full = np.concatenate(outs, axis=0)  # [64,254,254]
    return full.reshape(F, HO, WO, 1)


if __name__ == "__main__":
    nc = _build_bass()
    print("built ok")
